# revision 14
# baseline (speedup 1.0000x reference)
"""Trainium2 Bass kernel for nn_AttentionLayer_77558519431766.

Math: the reference computes softmax over a size-1 axis, which is
identically 1.0, so the attention MLP is dead code and

    out[b, e] = sum_{i<j} x[b,i,e] * x[b,j,e]
              = 0.5 * ((sum_f x[b,f,e])^2 - sum_f x[b,f,e]^2)

Design (v6, per 128-sample chunk, layout [128b, f*64+e], all f32):
  - Input DMA (HWDGE/sync, f32) in 4 column sub-slices per chunk,
    chunks back to back, so chunk0's tail hides under chunk1's stream.
  - Squares computed in the batch-major layout straight off the wire
    (DVE tensor_mul / ACT Square, split by slice so neither engine
    saturates).
  - PE runs ACCUMULATING f32 transposes (transpose = matmul, so
    start/stop accumulation applies): all 25 blocks of x sum into one
    [128,128] PSUM tile pt_s (partition (f%2, e) holds sum over f of
    that parity), and the 25 blocks of x^2 into pt_q. The f-reduction
    happens inside PSUM for free - PSUM->SBUF traffic drops from 410k
    elements per chunk to 2x16k.
  - Tail: copy pt_s/pt_q to SBUF, one mask matmul each (two-hot mask,
    f32, scaled sqrt(0.5)/0.5) yields s' and q' as [128b, 64e] in
    PSUM, then res = Square(s') - q' and a 32KB output DMA.
  - Two wide f32 dummy matmuls warm the PE clock gate (1.2 -> 2.4GHz)
    during the preamble/DMA fill.

Sharding: pure data parallelism, batch 2048 -> 8 shards of 256.
"""

import numpy as np

try:
    import concourse.bass as bass  # noqa: F401
except ImportError:  # pragma: no cover
    import sys

    sys.path.insert(0, "/opt/trn_rl_repo")

_B, _F, _E = 2048, 50, 64
_NCORES = 8
_BS = _B // _NCORES  # 256 rows per core
_ROW = _F * _E  # 3200 floats per row
_P = 128  # SBUF partitions

# (col0, col1, block0, block1, square-engine) per sub-slice
_SLICES = [
    (0, 1024, 0, 8, "act"),
    (1024, 2048, 8, 16, "dve"),
    (2048, 3072, 16, 24, "act"),
    (3072, 3200, 24, 25, "mix"),  # c0 -> act, c1 -> dve
]
_NBLK = 25
_NWARM = 2


def _make_tc_class():
    """TileContext with a slim kernel tail: keep only the global-clock
    drain; the NEFF exit barrier is runtime-emitted regardless."""
    from concourse.tile import TileContext
    from concourse.vector_clock import ScopedClock

    class SlimTailTileContext(TileContext):
        def _drain_and_barrier(self, tick_clock, wait_clock):
            drain_inst = self.nc.sync.drain()
            wait_clock.add_sem_waits(
                drain_inst.ins, ScopedClock({None: tick_clock.global_clock})
            )
            popped = self.nc._tile_sem_poison_stack.pop()
            assert popped is self._sem_poison

    return SlimTailTileContext


def _build():
    import concourse.bacc as bacc
    import concourse.mybir as mybir

    TileContext = _make_tc_class()

    f32 = mybir.dt.float32
    bf16 = mybir.dt.bfloat16
    i32 = mybir.dt.int32
    SQ = mybir.ActivationFunctionType.Square
    ALU = mybir.AluOpType
    HALF_SQRT = float(np.float32(np.sqrt(0.5)))

    nc = bacc.Bacc()
    x = nc.declare_dram_parameter("inputs", [_BS, _ROW], f32, isOutput=False)
    out = nc.declare_dram_parameter("out", [_BS, _E], f32, isOutput=True)

    n_chunks = _BS // _P  # 2
    rows_of = lambda c: slice(c * _P, (c + 1) * _P)

    with TileContext(nc) as tc:
        with (
            tc.tile_pool(name="consts", bufs=1) as cpool,
            tc.tile_pool(name="xt", bufs=2) as xtpool,
            tc.tile_pool(name="xsq", bufs=2) as sqpool,
            tc.tile_pool(name="pts", bufs=2, space="PSUM") as pspool,
            tc.tile_pool(name="ptq", bufs=2, space="PSUM") as pqpool,
            tc.tile_pool(name="sq2", bufs=1, space="PSUM") as sq2pool,
            tc.tile_pool(name="wp", bufs=1, space="PSUM") as wppool,
            tc.tile_pool(name="small", bufs=4) as spool,
        ):
            # All input DMAs first on sync, chunk-serial.
            xts, xsqs = [], []
            for c in range(n_chunks):
                xt = xtpool.tile([_P, _ROW], f32, tag="xt")
                xsq = sqpool.tile([_P, _ROW], f32, tag="xsq")
                xts.append(xt)
                xsqs.append(xsq)
            for c in range(n_chunks):
                for c0, c1, b0, b1, eng in _SLICES:
                    nc.sync.dma_start(
                        out=xts[c][:, c0:c1], in_=x[rows_of(c), c0:c1]
                    )

            # Constants: f32 identity + scaled two-hot masks; PE warm
            # source tile.
            wsrc = cpool.tile([_P, 512], f32, tag="wsrc")
            nc.gpsimd.memset(wsrc[:], 0.0)
            iot_i = cpool.tile([_P, _P], i32, tag="iot_i")
            iot_m = cpool.tile([_P, _E], i32, tag="iot_m")
            ident = cpool.tile([_P, _P], f32, tag="ident")
            mask_a = cpool.tile([_P, _E], bf16, tag="mask_a")
            mask_b = cpool.tile([_P, _E], bf16, tag="mask_b")
            maskh = cpool.tile([_P, _E], bf16, tag="maskh")
            maskq = cpool.tile([_P, _E], bf16, tag="maskq")
            nc.gpsimd.iota(iot_i[:], pattern=[[1, _P]], base=0, channel_multiplier=-1)
            nc.gpsimd.iota(iot_m[:], pattern=[[1, _E]], base=0, channel_multiplier=-1)
            nc.vector.tensor_scalar(ident[:], iot_i[:], 0, None, op0=ALU.is_equal)
            nc.vector.tensor_scalar(mask_a[:], iot_m[:], 0, None, op0=ALU.is_equal)
            nc.vector.tensor_scalar(mask_b[:], iot_m[:], -_E, None, op0=ALU.is_equal)
            nc.vector.tensor_add(mask_a[:], mask_a[:], mask_b[:])
            nc.vector.tensor_scalar_mul(maskh[:], mask_a[:], HALF_SQRT)
            nc.vector.tensor_scalar_mul(maskq[:], mask_a[:], 0.5)

            # PE warm-up: two wide f32 matmuls (512 moving cols each).
            wp = wppool.tile([_P, 512], f32, tag="wp")
            for i in range(_NWARM):
                nc.tensor.matmul(wp[:], wsrc[:, 0:_P], wsrc[:], start=True, stop=True)

            pts, ptqs = [], []
            for c in range(n_chunks):
                pt_s = pspool.tile([_P, _P], f32, tag="pt_s")
                pt_q = pqpool.tile([_P, _P], f32, tag="pt_q")
                pts.append(pt_s)
                ptqs.append(pt_q)

            def emit_chunk_stream(c):
                """Squares + accumulating transposes for chunk c."""
                xt, xsq = xts[c], xsqs[c]
                for c0, c1, b0, b1, eng in _SLICES:
                    if eng == "mix":
                        eng = "act" if c == 0 else "dve"
                    if eng == "act":
                        nc.scalar.activation(xsq[:, c0:c1], xt[:, c0:c1], SQ)
                    else:
                        nc.vector.tensor_mul(
                            xsq[:, c0:c1], xt[:, c0:c1], xt[:, c0:c1]
                        )
                    for k in range(b0, b1):
                        nc.tensor.matmul(
                            pts[c][:],
                            xt[:, k * _P : (k + 1) * _P],
                            ident[:],
                            is_transpose=True,
                            start=(k == 0),
                            stop=(k == _NBLK - 1),
                        )
                    for k in range(b0, b1):
                        nc.tensor.matmul(
                            ptqs[c][:],
                            xsq[:, k * _P : (k + 1) * _P],
                            ident[:],
                            is_transpose=True,
                            start=(k == 0),
                            stop=(k == _NBLK - 1),
                        )

            def emit_chunk_tail(c):
                u = spool.tile([_P, _P], bf16, tag=f"u_{c}")
                v = spool.tile([_P, _P], bf16, tag=f"v_{c}")
                m2 = spool.tile([_P, _E], f32, tag=f"m2_{c}")
                res = spool.tile([_P, _E], f32, tag=f"res_{c}")
                s2 = sq2pool.tile([_P, 2 * _E], f32, tag=f"sq_{c}")
                nc.vector.tensor_copy(u[:], pts[c][:])
                nc.vector.tensor_copy(v[:], ptqs[c][:])
                nc.tensor.matmul(
                    s2[:, 0:_E], u[:], maskh[:], start=True, stop=True
                )
                nc.tensor.matmul(
                    s2[:, _E : 2 * _E], v[:], maskq[:], start=False, stop=True
                )
                nc.scalar.activation(m2[:], s2[:, 0:_E], SQ)
                nc.vector.tensor_sub(res[:], m2[:], s2[:, _E : 2 * _E])
                nc.sync.dma_start(out=out[rows_of(c), :], in_=res[:])

            emit_chunk_stream(0)
            emit_chunk_stream(1)
            emit_chunk_tail(0)
            emit_chunk_tail(1)
    nc.compile()
    return nc


_WALRUS_EXTRA = []


def _patch_walrus():
    """Cap walrus's semaphore allocation (unused semaphores cost ~150ns
    each in the NEFF postamble)."""
    from concourse import bass_utils

    if getattr(bass_utils, "_walrus_patched", False):
        return
    real_run = bass_utils.run_command

    def run2(cmd, **kw):
        if cmd and "walrus_driver" in str(cmd[0]):
            cmd = list(cmd) + _WALRUS_EXTRA
        return real_run(cmd, **kw)

    bass_utils.run_command = run2
    bass_utils._walrus_patched = True


def _run(in_maps, **kwargs):
    from concourse.bass_utils import run_bass_kernel_spmd

    _patch_walrus()
    nc = _build()
    return run_bass_kernel_spmd(nc, in_maps, core_ids=list(range(_NCORES)), **kwargs)


def _shard(inputs: np.ndarray):
    x = np.ascontiguousarray(
        np.asarray(inputs, dtype=np.float32).reshape(_B, _ROW)
    )
    return [
        {"inputs": np.ascontiguousarray(x[i * _BS : (i + 1) * _BS])}
        for i in range(_NCORES)
    ]


def kernel(
    inputs: np.ndarray,
    weight_attention: np.ndarray = None,
    weight_projection: np.ndarray = None,
    weight_bias: np.ndarray = None,
) -> np.ndarray:
    # weights are dead code (softmax over a size-1 axis == 1.0)
    res = _run(_shard(inputs))
    return np.concatenate([r["out"] for r in res.results], axis=0)
